# revision 5
# baseline (speedup 1.0000x reference)
"""MoE ExpertGroup kernel for Trainium2 (8 NeuronCores, expert-parallel).

Problem: E=8 experts, H=1024, I=4096, N=16384 tokens sorted by expert.
y[t] = gelu_tanh(x[t] @ w1[e(t)]) @ w2[e(t)]

Sharding: expert-parallel — core e holds expert e's weights and processes
expert e's contiguous token block (balanced routing: 2048 tokens/core).

v2 design (bf16 matmuls, ~437us tensor-engine floor):
  - All matmul operands bf16 (host converts): separate LDWEIGHTS + FWL,
    halved DMA traffic. Accuracy sim: rel_err 3.2e-3 (gate 2e-2).
  - Weights fully SBUF-resident ([128, k, cols] host-packed layout so a
    single DMA fills a k-major chunk); x shipped pre-transposed the same
    way; DMAs are chunked and ordered so the first MM1 chain is fed after
    ~2MB instead of ~8MB.
  - MM2 accumulates the full K=4096 in one PSUM bank per (hh,tc) group —
    no ysb SBUF accumulation stage, no DVE adds.
  - PE warmup MMs sized to cover the first-wave DMA (HAM clock-gate).

Per-core dataflow, per 512-token block b (4 blocks):
  MM1: ph[128 I, 512 tok] += w1[:,k,il]^T @ xT[:,k,:]   (k=0..7)
       gelu -> ht[il] bf16                               (il=0..31)
  MM2: py[128 tok, 512 H] += ht[il][:,tc]^T @ w2[:,il,hs] (il=0..31)
       DVE copy -> y_sb -> DMA out                       (hh=0..1, tc=0..3)
"""

import sys

sys.path.insert(0, "/opt/trn_rl_repo")

import numpy as np
import ml_dtypes

# --- problem constants (hardcoded; kernel.py must be self-contained) ---
E = 8          # experts == cores
H = 1024       # hidden
I = 4096       # intermediate
N_TOK = 16384  # total tokens
T = N_TOK // E  # tokens per core (capacity)

P = 128
TB = 512             # token block (psum free dim)
NB = T // TB         # 4 blocks
KH = H // P          # 8 k-tiles (MM1 contraction)
IB = I // P          # 32 i-tiles (MM2 contraction)
WCH = 512            # w1 chunk width (I cols per DMA chunk)
NW1C = I // WCH      # 8 w1 chunks
W2G = 4              # w2 i-tiles per chunk
NW2C = IB // W2G     # 8 w2 chunks
N_WARM = 20          # PE warmup matmuls (cover first-wave DMA)

_CACHE = {}


def _build():
    import concourse.bacc as bacc
    import concourse.mybir as mybir
    import concourse.tile as tile

    F32 = mybir.dt.float32
    BF16 = mybir.dt.bfloat16
    GELU = mybir.ActivationFunctionType.Gelu_apprx_tanh

    nc = bacc.Bacc("TRN2", target_bir_lowering=False, debug=False, num_devices=E)

    # host-packed layouts: [128 partitions, k-tile, cols]
    xd = nc.dram_tensor("xT", [P, KH, T], BF16, kind="ExternalInput").ap()
    w1d = nc.dram_tensor("w1", [P, KH, I], BF16, kind="ExternalInput").ap()
    w2d = nc.dram_tensor("w2", [P, IB, H], BF16, kind="ExternalInput").ap()
    y = nc.dram_tensor("y", [T, H], F32, kind="ExternalOutput").ap()

    with tile.TileContext(nc) as tc:
        with (
            tc.tile_pool(name="wsrc", bufs=1) as wsrc_pool,
            tc.tile_pool(name="w1p", bufs=1) as w1_pool,
            tc.tile_pool(name="w2p", bufs=1) as w2_pool,
            tc.tile_pool(name="xp", bufs=1) as x_pool,
            tc.tile_pool(name="htp", bufs=1) as ht_pool,
            tc.tile_pool(name="yp", bufs=4) as y_pool,
            tc.tile_pool(name="ph", bufs=4, space="PSUM") as ph_pool,
            tc.tile_pool(name="py", bufs=4, space="PSUM") as py_pool,
        ):
            # --- PE warmup: release the HAM clock gate while DMAs land ---
            wsrc = wsrc_pool.tile([P, TB], BF16, tag="warm", name="wsrc")
            nc.gpsimd.memset(wsrc[:], 0.0)
            for _ in range(N_WARM):
                pw = ph_pool.tile([P, TB], F32, tag="ph", name="pw")
                nc.tensor.matmul(pw[:], wsrc[:, :P], wsrc[:], start=True, stop=True)

            # --- input DMAs, priority-ordered ---
            # w1 chunk c holds I-cols [c*512, (c+1)*512) for all 8 k-tiles
            w1c = [
                w1_pool.tile([P, KH, WCH], BF16, tag=f"w1c{c}", name=f"w1c{c}")
                for c in range(NW1C)
            ]
            # x block tiles (double-buffered by parity)
            xb = [
                x_pool.tile([P, KH, TB], BF16, tag=f"xb{par}", name=f"xb{par}")
                for par in range(2)
            ]
            # first wave: w1 chunk 0, then x block 0 in two k-halves
            nc.sync.dma_start(out=w1c[0][:], in_=w1d[:, :, 0:WCH])
            nc.sync.dma_start(out=xb[0][:, 0:4, :], in_=xd[:, 0:4, 0:TB])
            nc.sync.dma_start(out=xb[0][:, 4:8, :], in_=xd[:, 4:8, 0:TB])
            # rest of w1
            for c in range(1, NW1C):
                nc.sync.dma_start(
                    out=w1c[c][:], in_=w1d[:, :, c * WCH : (c + 1) * WCH]
                )
            # x block 1
            nc.sync.dma_start(out=xb[1][:], in_=xd[:, :, TB : 2 * TB])
            # w2 chunks (needed from ~55us in)
            w2c = [
                w2_pool.tile([P, W2G, H], BF16, tag=f"w2c{c}", name=f"w2c{c}")
                for c in range(NW2C)
            ]
            for c in range(NW2C):
                nc.sync.dma_start(
                    out=w2c[c][:], in_=w2d[:, c * W2G : (c + 1) * W2G, :]
                )

            ht = [
                ht_pool.tile([P, TB], BF16, tag=f"ht{il}", name=f"ht{il}")
                for il in range(IB)
            ]

            for b in range(NB):
                xcur = xb[b % 2]

                # MM1 + gelu
                for il in range(IB):
                    ph = ph_pool.tile([P, TB], F32, tag="ph", name="ph")
                    wt = w1c[il // 4]
                    ic = (il % 4) * P
                    for k in range(KH):
                        nc.tensor.matmul(
                            ph[:],
                            wt[:, k, ic : ic + P],
                            xcur[:, k, :],
                            start=(k == 0),
                            stop=(k == KH - 1),
                        )
                    nc.scalar.activation(ht[il][:], ph[:], GELU)

                # prefetch x block b+2 into the parity buffer MM1(b) just
                # finished reading (WAR dep delays the transfer until then)
                if b + 2 < NB:
                    nc.sync.dma_start(
                        out=xb[b % 2][:],
                        in_=xd[:, :, (b + 2) * TB : (b + 3) * TB],
                    )

                # MM2: accumulate full K=4096 per (hh, tc) group
                for hh in range(2):
                    hs = slice(hh * (H // 2), (hh + 1) * (H // 2))
                    for tc_ in range(TB // P):
                        py = py_pool.tile([P, H // 2], F32, tag="py", name="py")
                        for il in range(IB):
                            nc.tensor.matmul(
                                py[:],
                                ht[il][:, tc_ * P : (tc_ + 1) * P],
                                w2c[il // W2G][:, il % W2G, hs],
                                start=(il == 0),
                                stop=(il == IB - 1),
                            )
                        ysb = y_pool.tile([P, H // 2], F32, tag="ysb", name="ysb")
                        nc.vector.tensor_copy(ysb[:], py[:])
                        nc.sync.dma_start(
                            out=y[b * TB + tc_ * P : b * TB + (tc_ + 1) * P, hs],
                            in_=ysb[:],
                        )

    nc.compile()
    return nc


def _get_nc():
    if "nc" not in _CACHE:
        _CACHE["nc"] = _build()
    return _CACHE["nc"]


def _pack_k(a, ktiles):
    """[R, C] with R = ktiles*128 -> [128, ktiles, C] bf16 contiguous."""
    r, c = a.shape
    assert r == ktiles * P
    return np.ascontiguousarray(
        a.reshape(ktiles, P, c).transpose(1, 0, 2).astype(ml_dtypes.bfloat16)
    )


def _prep(x_sorted, w1, w2, expert_counts):
    x_sorted = np.ascontiguousarray(x_sorted, dtype=np.float32)
    w1 = np.asarray(w1, dtype=np.float32)
    w2 = np.asarray(w2, dtype=np.float32)
    counts = np.asarray(expert_counts, dtype=np.int64)

    n = x_sorted.shape[0]
    offsets = np.cumsum(counts)
    # per-token expert id, identical to reference's searchsorted
    eid = np.searchsorted(offsets, np.arange(n), side="right")

    in_maps = []
    row_idx = []
    for e in range(E):
        rows = np.nonzero(eid == e)[0]
        assert len(rows) <= T, f"expert {e} overflows capacity {T}"
        xe = np.zeros((T, H), dtype=np.float32)
        xe[: len(rows)] = x_sorted[rows]
        row_idx.append(rows)
        in_maps.append(
            {
                "xT": _pack_k(np.ascontiguousarray(xe.T), KH),
                "w1": _pack_k(w1[e], KH),
                "w2": _pack_k(w2[e], IB),
            }
        )
    return in_maps, row_idx


def kernel(x_sorted, w1, w2, expert_counts, local_expert_indices, **_unused):
    from concourse.bass_utils import run_bass_kernel_spmd

    n = np.asarray(x_sorted).shape[0]
    in_maps, row_idx = _prep(x_sorted, w1, w2, expert_counts)
    nc = _get_nc()

    res = run_bass_kernel_spmd(nc, in_maps, list(range(E))).results

    out = np.zeros((n, H), dtype=np.float32)
    for e in range(E):
        rows = row_idx[e]
        out[rows] = res[e]["y"][: len(rows)]
    return out


# revision 9
# speedup vs baseline: 1.1217x; 1.1217x over previous
"""MoE ExpertGroup kernel for Trainium2 (8 NeuronCores, expert-parallel).

Problem: E=8 experts, H=1024, I=4096, N=16384 tokens sorted by expert.
y[t] = gelu_tanh(x[t] @ w1[e(t)]) @ w2[e(t)]

Sharding: expert-parallel — core e holds expert e's weights and processes
expert e's contiguous token block (balanced routing: 2048 tokens/core).

v3: fp32r matmuls (measured 227ns/MM pace vs 259 for bf16 — the bf16
separate-LDWEIGHTS path serializes ~46ns/MM while fp32r's internal
weight load overlaps).  Structural wins vs the original baseline:
  - host-packed k-major DRAM layouts ([128, ktile, cols]) so each w1/w2
    group chunk and x half-chunk is ONE dma_start (52 total vs 240);
    priority-ordered so the first MM1 chain is fed after ~4MB.
  - warmup matmul count tuned to cover the first-wave DMA (HAM gate).
  - MM2 interleaves the two H-half accumulation chains so consecutive
    matmuls share the same stationary hT tile.
  - y written out per (token-tile, H-half) right after its last add.

Per-core dataflow per half (2 halves x 1024 tokens), per group g (8
groups x 4 I-tiles), all matmuls fp32r:
  MM1: ph[128 I, 512 tok] += w1c[:,k,il]^T @ xT[:,k,tb]  (k=0..7)
       gelu -> hT[il]                                     (il=0..3)
  MM2: py[hh][128 tok, 512 H] += hT[il][:,tc]^T @ w2c[:,il,hh]
       (il chains for hh=0,1 interleaved); DVE-accumulate into ysb
  last group: ysb[tt][:,hh] -> DMA out
"""

import sys

sys.path.insert(0, "/opt/trn_rl_repo")

import numpy as np

# --- problem constants (hardcoded; kernel.py must be self-contained) ---
E = 8          # experts == cores
H = 1024       # hidden
I = 4096       # intermediate
N_TOK = 16384  # total tokens
T = N_TOK // E  # tokens per core (capacity)

P = 128
NH = 2               # token halves per core
TH = T // NH         # tokens per half (1024)
TB = 512             # token block (psum free dim)
NTB = TH // TB       # 2
KH = H // P          # 8
IB = I // P          # 32
GI = 4               # I-tiles per PSUM-accumulation group
NG = IB // GI        # 8 groups
N_WARM = 10          # PE warmup matmuls (cover first-wave DMA)

_CACHE = {}


def _build():
    import concourse.bacc as bacc
    import concourse.mybir as mybir
    import concourse.tile as tile

    F32 = mybir.dt.float32
    F32R = mybir.dt.float32r
    GELU = mybir.ActivationFunctionType.Gelu_apprx_tanh

    nc = bacc.Bacc("TRN2", target_bir_lowering=False, debug=False, num_devices=E)

    # host-packed layouts: [128 partitions, k-tile, cols]
    xd = nc.dram_tensor("xT", [P, KH, T], F32R, kind="ExternalInput").ap()
    w1d = nc.dram_tensor("w1", [P, KH, I], F32R, kind="ExternalInput").ap()
    w2d = nc.dram_tensor("w2", [P, IB, H], F32R, kind="ExternalInput").ap()
    y = nc.dram_tensor("y", [T, H], F32, kind="ExternalOutput").ap()

    with tile.TileContext(nc) as tc:
        with (
            tc.tile_pool(name="wsrc", bufs=1) as wsrc_pool,
            tc.tile_pool(name="xp", bufs=1) as x_pool,
            tc.tile_pool(name="ysb", bufs=1) as y_pool,
            tc.tile_pool(name="w1p", bufs=2) as w1_pool,
            tc.tile_pool(name="w2p", bufs=2) as w2_pool,
            tc.tile_pool(name="hT", bufs=8) as hT_pool,
            tc.tile_pool(name="ph", bufs=4, space="PSUM") as ph_pool,
            tc.tile_pool(name="py", bufs=4, space="PSUM") as py_pool,
        ):
            # PE warmup: release the HAM clock gate while the first DMAs land
            wsrc = wsrc_pool.tile([P, TB], F32, tag="warm", name="wsrc")
            nc.gpsimd.memset(wsrc[:], 0.0)
            for _ in range(N_WARM):
                pw = ph_pool.tile([P, TB], F32, tag="ph", name="pw")
                nc.tensor.matmul(pw[:], wsrc[:, :P], wsrc[:], start=True, stop=True)

            def fetch_w(g):
                # one dma per w1 group chunk ([128, 8, 512] = 2MB)
                w1t = w1_pool.tile([P, KH, GI * P], F32R, tag="w1c", name="w1c")
                nc.sync.dma_start(
                    out=w1t[:], in_=w1d[:, :, g * GI * P : (g + 1) * GI * P]
                )
                # one dma per w2 group chunk ([128, 4, 1024] = 2MB)
                w2t = w2_pool.tile([P, GI, H], F32R, tag="w2c", name="w2c")
                nc.sync.dma_start(out=w2t[:], in_=w2d[:, g * GI : (g + 1) * GI, :])
                return w1t, w2t

            for half in range(NH):
                t0 = half * TH

                # priority order: w1 g0 first, then xT chunk a (the first
                # MM1 chain needs exactly these two), then w2 g0, xT chunk b
                xt = x_pool.tile([P, KH, TH], F32R, tag=f"xT{half}", name=f"xT{half}")
                w1t0 = w1_pool.tile([P, KH, GI * P], F32R, tag="w1c", name="w1c")
                nc.sync.dma_start(out=w1t0[:], in_=w1d[:, :, 0 : GI * P])
                nc.sync.dma_start(
                    out=xt[:, :, 0:TB], in_=xd[:, :, t0 : t0 + TB]
                )
                w2t0 = w2_pool.tile([P, GI, H], F32R, tag="w2c", name="w2c")
                nc.sync.dma_start(out=w2t0[:], in_=w2d[:, 0:GI, :])
                nc.sync.dma_start(
                    out=xt[:, :, TB : 2 * TB], in_=xd[:, :, t0 + TB : t0 + 2 * TB]
                )

                ysb = [
                    y_pool.tile([P, H], F32, tag=f"yt{tt}", name=f"yt{tt}")
                    for tt in range(TH // P)
                ]

                for g in range(NG):
                    w1t, w2t = (w1t0, w2t0) if g == 0 else fetch_w(g)

                    for tb in range(NTB):
                        ts_ = slice(tb * TB, (tb + 1) * TB)
                        hTt = []
                        for il in range(GI):
                            ph = ph_pool.tile([P, TB], F32, tag="ph", name="ph")
                            for k in range(KH):
                                nc.tensor.matmul(
                                    ph[:],
                                    w1t[:, k, il * P : (il + 1) * P],
                                    xt[:, k, ts_],
                                    start=(k == 0),
                                    stop=(k == KH - 1),
                                )
                            ht = hT_pool.tile([P, TB], F32R, tag="ht", name="ht")
                            nc.scalar.activation(ht[:], ph[:], GELU)
                            hTt.append(ht)
                        for tc_ in range(TB // P):
                            tt = tb * (TB // P) + tc_
                            # interleave the two H-half chains: consecutive
                            # matmuls share the same stationary hT slice
                            py0 = py_pool.tile([P, H // 2], F32, tag="py", name="py")
                            py1 = py_pool.tile([P, H // 2], F32, tag="py", name="py")
                            for il in range(GI):
                                lhs = hTt[il][:, tc_ * P : (tc_ + 1) * P]
                                nc.tensor.matmul(
                                    py0[:], lhs, w2t[:, il, 0 : H // 2],
                                    start=(il == 0), stop=(il == GI - 1),
                                )
                                nc.tensor.matmul(
                                    py1[:], lhs, w2t[:, il, H // 2 : H],
                                    start=(il == 0), stop=(il == GI - 1),
                                )
                            for hh, py in ((0, py0), (1, py1)):
                                hs = slice(hh * (H // 2), (hh + 1) * (H // 2))
                                if g == 0:
                                    nc.scalar.activation(
                                        ysb[tt][:, hs], py[:],
                                        mybir.ActivationFunctionType.Copy,
                                    )
                                else:
                                    nc.vector.tensor_add(
                                        ysb[tt][:, hs], ysb[tt][:, hs], py[:]
                                    )
                                if g == NG - 1:
                                    nc.sync.dma_start(
                                        out=y[t0 + tt * P : t0 + (tt + 1) * P, hs],
                                        in_=ysb[tt][:, hs],
                                    )

    nc.compile()
    return nc


def _get_nc():
    if "nc" not in _CACHE:
        _CACHE["nc"] = _build()
    return _CACHE["nc"]


def _pack_k(a, ktiles):
    """[R, C] with R = ktiles*128 -> [128, ktiles, C] f32 contiguous."""
    r, c = a.shape
    assert r == ktiles * P
    return np.ascontiguousarray(
        a.reshape(ktiles, P, c).transpose(1, 0, 2).astype(np.float32)
    )


def _prep(x_sorted, w1, w2, expert_counts):
    x_sorted = np.ascontiguousarray(x_sorted, dtype=np.float32)
    w1 = np.asarray(w1, dtype=np.float32)
    w2 = np.asarray(w2, dtype=np.float32)
    counts = np.asarray(expert_counts, dtype=np.int64)

    n = x_sorted.shape[0]
    offsets = np.cumsum(counts)
    # per-token expert id, identical to reference's searchsorted
    eid = np.searchsorted(offsets, np.arange(n), side="right")

    in_maps = []
    row_idx = []
    for e in range(E):
        rows = np.nonzero(eid == e)[0]
        assert len(rows) <= T, f"expert {e} overflows capacity {T}"
        xe = np.zeros((T, H), dtype=np.float32)
        xe[: len(rows)] = x_sorted[rows]
        row_idx.append(rows)
        in_maps.append(
            {
                "xT": _pack_k(np.ascontiguousarray(xe.T), KH),
                "w1": _pack_k(w1[e], KH),
                "w2": _pack_k(w2[e], IB),
            }
        )
    return in_maps, row_idx


def kernel(x_sorted, w1, w2, expert_counts, local_expert_indices, **_unused):
    from concourse.bass_utils import run_bass_kernel_spmd

    n = np.asarray(x_sorted).shape[0]
    in_maps, row_idx = _prep(x_sorted, w1, w2, expert_counts)
    nc = _get_nc()

    res = run_bass_kernel_spmd(nc, in_maps, list(range(E))).results

    out = np.zeros((n, H), dtype=np.float32)
    for e in range(E):
        rows = row_idx[e]
        out[rows] = res[e]["y"][: len(rows)]
    return out
